# revision 32
# baseline (speedup 1.0000x reference)
"""Trainium2 Bass kernel for the NeuralMemory (scatter_memory) problem.

Math (B=1, N=512, D=128, DEPTH=4): per-token meta-gradients of the memory
MLP are rank-1 per layer, so the (n, depth, d, d) momentum/update scans
collapse to a scalar coefficient matrix C[t,s] applied attention-style:

    retrieved_l(t) = y_t @ W_l + sum_s C[t,s]*(-lr_s) * (y_t . x_l(s)) * g_l(s)

The recurrence coefficients decay geometrically (|am| ~ 0.23, (1-decay) ~ 0.5),
so C is numerically banded: C[t,s] == 0 (fp32) for t-s >= 64.  Each of the 8
cores therefore handles one 64-query window [qc, qc+64) and only needs the
128-token key window [qc-64, qc+64) — fully data-parallel, no collectives.
Core 0's missing past is zero-padded on the host (zero keys/lr make those
contributions vanish identically).

Per core everything is a single (128,128) tile:
  - C^T (with -2/D*lr folded into the scan impulse) is built exactly with two
    hardware linear-recurrence scans (tensor_tensor_scan).
  - row->all-partition broadcasts (lr/mom/decay) are done by matmuls against
    column-replicated weight matrices uploaded from the host (no GPSIMD).
  - the only ACT table set used is silu_and_others (Silu + Tanh); derivative
    silu is computed on DVE from tanh: s=(1+th)/2, sp = s + x - x*s.
"""

import numpy as np

D = 128
N = 512
DEPTH = 4
NCORES = 8
QW = N // NCORES        # 64 queries per core
SW = 2 * QW             # 128-token key window per core

# column offsets inside the consolidated per-core input tensor (128, ALLW).
# Part A [0:1024) carries everything the forward pass needs first.
OFF_SEQW = 0                     # (128, 128) seq^T window [qc-64, qc+64)
OFF_WQ = 128                     # (128, 128)
OFF_WK = 256
OFF_WM0 = 384                    # W_mem layer 0
OFF_REP = 512                    # 3 x (128, 128): lr*(-2/D) rep, mom rep, dec rep
OFF_ID = 896                     # (128, 128) identity (impulse + transposes)
OFF_WV = 1024
OFF_WM123 = 1152                 # W_mem layers 1..3
OFF_WMT = 1536                   # 4 x (128, 128) W_mem layers transposed
ALLW = 2048

_cache = {}


def _build_program():
    import concourse.mybir as mybir
    from concourse import bacc
    from concourse.tile import TileContext

    f32 = mybir.dt.float32
    fp16 = mybir.dt.float16
    AF = mybir.ActivationFunctionType
    ALU = mybir.AluOpType

    nc = bacc.Bacc("TRN2")

    allin_d = nc.dram_tensor("allin", [D, ALLW], fp16, kind="ExternalInput")
    outT_d = nc.dram_tensor("outT", [D, QW], fp16, kind="ExternalOutput")

    with TileContext(nc) as tc:
        with (
            tc.tile_pool(name="sb", bufs=1) as sb,
            tc.tile_pool(name="ps", bufs=4, space="PSUM") as ps_pool,
            tc.tile_pool(name="ps_h", bufs=3, space="PSUM") as ps_h_pool,
            tc.tile_pool(name="ps_r", bufs=1, space="PSUM") as ps_r,
        ):
            def sbt(tag, shape=(D, SW), dt=fp16):
                return sb.tile(list(shape), dt, tag=tag, name=tag)

            def pst(tag, shape=(D, SW), dt=f32, pool=None):
                return (pool or ps_pool).tile(list(shape), dt, tag="ps",
                                              name=tag)

            allin = sbt("allin", (D, ALLW))
            # transfers serialize on one HW queue, so issue in first-use
            # order: [seq|wq|wk|wm0] releases the forward pass when it lands
            nc.sync.dma_start(out=allin[:, 0:512], in_=allin_d[:, 0:512])
            nc.sync.dma_start(out=allin[:, 512:1024], in_=allin_d[:, 512:1024])
            nc.sync.dma_start(out=allin[:, 1024:1536],
                              in_=allin_d[:, 1024:1536])
            nc.sync.dma_start(out=allin[:, 1536:2048], in_=allin_d[:, 1536:2048])

            # force the ACT table loads to happen during the input-DMA wait
            dum = sbt("dum", (1, 8))
            nc.gpsimd.memset(dum, 0.0)
            dum2 = sbt("dum2", (1, 8))
            nc.scalar.activation(dum2, dum, AF.Silu)

            seqW = allin[:, OFF_SEQW:OFF_SEQW + SW]
            wq = allin[:, OFF_WQ:OFF_WQ + D]
            wk = allin[:, OFF_WK:OFF_WK + D]
            wv = allin[:, OFF_WV:OFF_WV + D]
            rep_lr = allin[:, OFF_REP:OFF_REP + D]
            rep_mom = allin[:, OFF_REP + D:OFF_REP + 2 * D]
            rep_dec = allin[:, OFF_REP + 2 * D:OFF_REP + 3 * D]
            wm = [allin[:, OFF_WM0:OFF_WM0 + D]] + [
                allin[:, OFF_WM123 + D * l:OFF_WM123 + D * (l + 1)]
                for l in range(DEPTH - 1)]
            wmT = [allin[:, OFF_WMT + D * l:OFF_WMT + D * (l + 1)]
                   for l in range(DEPTH)]
            idm = allin[:, OFF_ID:OFF_ID + D]

            # ---- key projection first: it gates the whole forward chain ----
            ps_x0 = pst("x0")
            nc.tensor.matmul(ps_x0, wk, seqW, start=True, stop=True)
            x0 = sbt("x0")
            nc.vector.tensor_copy(x0, ps_x0)

            # ---- store forward immediately; the projection/broadcast
            # matmuls are interleaved one-per-gap so the PE queue stays in
            # execution order (head-of-line!).  Layer 3's tanh goes first
            # because sp_3 gates the backward chain.
            # sp_l = Dsilu(h_l) = s + x - x*s with s = (1+tanh(h/2))/2
            X = [x0]
            TH = [None]
            ps_q = pst("q", (D, QW))
            ps_v = pst("v")
            ps_lrb = pst("lrb")
            ps_amb = pst("amb")
            ps_dec = pst("dec")
            for l in range(DEPTH):
                ps_h = pst(f"h{l}", pool=ps_h_pool if l < DEPTH - 1 else None)
                nc.tensor.matmul(ps_h, wm[l], X[l], start=True, stop=True)
                # fillers: run in the PE gap while ACT does silu/tanh
                if l == 0:
                    nc.tensor.matmul(ps_q, wq, seqW[:, QW:SW],
                                     start=True, stop=True)
                elif l == 1:
                    nc.tensor.matmul(ps_v, wv, seqW, start=True, stop=True)
                    nc.tensor.matmul(ps_lrb, rep_lr, seqW,
                                     start=True, stop=True)
                elif l == 2:
                    nc.tensor.matmul(ps_amb, rep_mom, seqW,
                                     start=True, stop=True)
                    nc.tensor.matmul(ps_dec, rep_dec, seqW,
                                     start=True, stop=True)
                if l < DEPTH - 1:
                    xl = sbt(f"x{l + 1}")
                    thl = sbt(f"th{l + 1}")
                    if l == DEPTH - 2:
                        nc.scalar.activation(thl, ps_h, AF.Tanh, scale=0.5)
                        nc.scalar.activation(xl, ps_h, AF.Silu)
                    else:
                        nc.scalar.activation(xl, ps_h, AF.Silu)
                        nc.scalar.activation(thl, ps_h, AF.Tanh, scale=0.5)
                    X.append(xl)
                    TH.append(thl)
                else:
                    ps_h4 = ps_h

            qT = sbt("qT", (D, QW))
            nc.vector.tensor_copy(qT, ps_q)
            vT = sbt("vT")
            nc.vector.tensor_copy(vT, ps_v)

            # bb = 1 - sigmoid(dec) = 0.5 - 0.5*tanh(dec/2)
            th_dec = sbt("th_dec")
            nc.scalar.activation(th_dec, ps_dec, AF.Tanh, scale=0.5)
            bb = sbt("bb")
            nc.vector.tensor_scalar(bb, th_dec, -0.5, 0.5, ALU.mult, ALU.add)

            # impulse carrying -2/D*lr_s on the diagonal
            izlr = sbt("izlr")
            nc.vector.tensor_mul(izlr, idm, ps_lrb)

            # ---- first-level scan ----
            AT = sbt("AT")
            nc.vector.tensor_tensor_scan(AT, ps_amb, izlr, 0.0,
                                         ALU.mult, ALU.add)

            # sp_l = Dsilu(h_l) = s + x - x*s with s = (1+tanh(h/2))/2.
            # sp_3 and sp_1 on GpSimd, sp_2 on DVE — ordered so each is
            # ready when its backward layer needs it.
            SP = [None] * DEPTH

            def sp_chain(l, eng):
                sl = sbt(f"s{l}")
                eng.tensor_scalar(sl, TH[l], 0.5, 0.5, ALU.mult, ALU.add)
                xs = sbt(f"xs{l}")
                eng.tensor_mul(xs, X[l], sl)
                u = sbt(f"u{l}")
                eng.tensor_sub(u, X[l], xs)
                spl = sbt(f"sp{l}")
                eng.tensor_add(spl, sl, u)
                SP[l] = spl

            sp_chain(3, nc.gpsimd)
            sp_chain(2, nc.vector)
            sp_chain(1, nc.gpsimd)

            d4 = sbt("d4")
            nc.vector.tensor_sub(d4, ps_h4, vT)

            # ---- backward deltas; each G transpose follows its delta.
            # The second-level scan (C^T) is interleaved after d2 so it
            # doesn't block the backward chain on DVE (needed only by the
            # retrieval's first ct-multiply).
            G = [None] * DEPTH
            Dl = [None] * (DEPTH + 1)
            Dl[4] = d4
            CT = sbt("CT")
            for l in range(DEPTH - 1, -1, -1):
                ps_t = pst(f"t{l}", (D, D), dt=fp16)
                nc.tensor.transpose(ps_t, Dl[l + 1], idm)
                gl = sbt(f"g{l}")
                nc.scalar.copy(gl, ps_t)
                G[l] = gl
                if l > 0:
                    ps_b = pst(f"b{l}")
                    nc.tensor.matmul(ps_b, wmT[l], Dl[l + 1], start=True,
                                     stop=True)
                    dl = sbt(f"d{l}")
                    nc.vector.tensor_mul(dl, ps_b, SP[l])
                    Dl[l] = dl
                if l == 2:
                    nc.vector.tensor_tensor_scan(CT, bb, AT, 0.0,
                                                 ALU.mult, ALU.add)

            # ---- retrieval over this core's 64-query window ----
            Y = qT
            CTq = CT[:, QW:SW]
            for l in range(DEPTH):
                ps_s = pst(f"S{l}", (D, QW))
                nc.tensor.matmul(ps_s, X[l], Y, start=True, stop=True)
                cst = sbt(f"cst{l}", (D, QW))
                nc.vector.tensor_mul(cst, ps_s, CTq)
                ps_o = pst(f"r{l}", (D, QW), pool=ps_r)
                nc.tensor.matmul(ps_o, wm[l], Y, start=True, stop=False)
                nc.tensor.matmul(ps_o, G[l], cst, start=False, stop=True)
                if l < DEPTH - 1:
                    ynext = sbt(f"y{l + 1}", (D, QW))
                    nc.scalar.activation(ynext, ps_o, AF.Silu)
                    Y = ynext
                else:
                    outT = sbt("outT", (D, QW), dt=fp16)
                    nc.vector.tensor_copy(outT, ps_o)

            nc.sync.dma_start(out=outT_d[:, :], in_=outT)

    return nc


def get_program():
    if "nc" not in _cache:
        nc = _build_program()
        nc.finalize()
        _cache["nc"] = nc
    return _cache["nc"]


def make_in_maps(seq, W_mem, W_q, W_kv, W_mom, W_step, W_decay):
    seq = np.asarray(seq, dtype=np.float32)
    W_mem = np.asarray(W_mem, dtype=np.float32)
    W_kv = np.asarray(W_kv, dtype=np.float32)
    seqT = seq.reshape(N, D).T  # (d, n)

    base = np.zeros((D, ALLW), dtype=np.float16)
    base[:, OFF_WQ:OFF_WQ + D] = np.asarray(W_q, dtype=np.float32)
    base[:, OFF_WK:OFF_WK + D] = W_kv[:, :D]
    base[:, OFF_WV:OFF_WV + D] = W_kv[:, D:]
    lr_col = np.asarray(W_step, dtype=np.float32)[:, 0] * (-2.0 / D)
    base[:, OFF_REP:OFF_REP + D] = np.repeat(lr_col[:, None], D, axis=1)
    base[:, OFF_REP + D:OFF_REP + 2 * D] = np.repeat(
        np.asarray(W_mom, dtype=np.float32)[:, :1], D, axis=1)
    base[:, OFF_REP + 2 * D:OFF_REP + 3 * D] = np.repeat(
        np.asarray(W_decay, dtype=np.float32)[:, :1], D, axis=1)
    base[:, OFF_WM0:OFF_WM0 + D] = W_mem[0]
    for l in range(1, DEPTH):
        base[:, OFF_WM123 + D * (l - 1):OFF_WM123 + D * l] = W_mem[l]
    for l in range(DEPTH):
        base[:, OFF_WMT + D * l:OFF_WMT + D * (l + 1)] = W_mem[l].T
    base[:, OFF_ID:OFF_ID + D] = np.eye(D, dtype=np.float32)

    in_maps = []
    for c in range(NCORES):
        allin = base.copy()
        qc = c * QW
        lo = qc - QW
        win = np.zeros((D, SW), dtype=np.float16)
        src_lo = max(lo, 0)
        win[:, src_lo - lo:] = seqT[:, src_lo:qc + QW].astype(np.float16)
        allin[:, OFF_SEQW:OFF_SEQW + SW] = win
        in_maps.append({"allin": allin})
    return in_maps


def assemble(results):
    out = np.empty((N, D), dtype=np.float32)
    for c in range(NCORES):
        out[c * QW:(c + 1) * QW, :] = results[c]["outT"].T.astype(np.float32)
    return out.reshape(1, N, D)


def kernel(**inputs) -> np.ndarray:
    from concourse.bass_utils import run_bass_kernel_spmd

    nc = get_program()
    in_maps = make_in_maps(**inputs)
    res = run_bass_kernel_spmd(nc, in_maps, list(range(NCORES)))
    return assemble(res.results)
